# revision 44
# baseline (speedup 1.0000x reference)
"""Trainium2 Bass kernel for nn_GATQueryProjector (2-layer GAT, output = node 0's row).

The reference returns only h[0] -- node 0's layer-2 GAT output. The exact
computation reduces to node 0's 2-hop neighborhood: E2 in-edges at layer 2
(dsts = node 0), whose sources S1 need layer-1 outputs, which need the E1
in-edges of S1. Host code does index work only (subgraph discovery, gathers,
packing); every NeuronCore runs the full floating-point computation
redundantly (no collectives -- the node feature table is "replicated" per the
sharding hint, and the pruned problem is tiny).

Device program (per core):
  hET[f,e]   = W1^T x[src_e]     24 bf16 matmuls, edge dim free (=E1), the
                                 per-edge score matmuls interleaved lag-one
  sT[e,h]    = attA gather       src scores direct; dst scores from the
                                 self-loop columns of hET + a dselT matmul
  softmax    = leaky/exp + 0/1-selection matmuls (den, gather) on the PE
  hE[e,f]    via XBAR transpose-DMAs (f0-f2, hidden under the GEMM) and a
                                 PE transpose for the late f3 chunk
  out1T[f,s] = sum_e w_e hE[e,f]; relu(+b1); g = out1 @ W2; layer-2
               attention over E2 edges; final weighted row + b2.
HW notes: gpsimd must stay SBUF-only; max one PSUM operand per DVE op;
no stride-0 broadcast APs; no divide ALU (reciprocal+mult instead).
"""

import numpy as np

import concourse.bacc as bacc
import concourse.mybir as mybir
import concourse.tile as tile
from concourse import bass
from concourse.bass_utils import run_bass_kernel_spmd

N_CORES = 8
NEG_SLOPE = 0.2
P = 128
BF16 = mybir.dt.bfloat16
F32 = mybir.dt.float32


def build_data(x, edge_index, W1, a_src1, a_dst1, b1, W2, a_src2, a_dst2, b2):
    """Host-side index work: node 0's 2-hop subgraph + packed device inputs."""
    x = np.asarray(x, dtype=np.float32)
    src0, dst0 = edge_index[0], edge_index[1]
    # layer-2 in-edges of node 0 (+ self-loop, as reference appends)
    e2_src = src0[dst0 == 0]
    L2_src = np.concatenate([e2_src, np.array([0], dtype=src0.dtype)])
    S1 = np.unique(L2_src)  # sorted 1-hop in-neighbors of 0 (incl 0)
    S = len(S1)
    # layer-1 in-edges of every v in S1 (+ self-loops, appended LAST in S1 order)
    m1 = np.isin(dst0, S1)
    u1, v1 = src0[m1], dst0[m1]
    L1_src = np.concatenate([u1, S1])
    L1_dst = np.concatenate([v1, S1])
    E1 = len(L1_src)
    E2 = len(L2_src)
    assert S <= 128 and E2 <= 128 and E1 <= 512, (S, E2, E1)
    padn = (P - E1 if E1 < P else E1 % 2)  # >=128 edges, even count
    if padn:
        L1_src = np.concatenate([L1_src, np.repeat(L1_src[-1:], padn)])
        L1_dst = np.concatenate(
            [L1_dst, np.full(padn, -1, dtype=L1_dst.dtype)])
        E1 += padn

    s1pos = {int(v): i for i, v in enumerate(S1)}
    d1 = np.array([s1pos.get(int(v), -1) for v in L1_dst])  # dst slot per edge
    s2 = np.array([s1pos[int(u)] for u in L2_src])  # src slot per layer-2 edge

    H, Dh = a_src1.shape
    F1 = H * Dh
    IN_DIM = x.shape[1]
    OUT = W2.shape[1]
    KIN = IN_DIM // P
    FH = F1 // P
    Sp = S + (S % 2)  # dsel free width (even)

    bf = lambda a: np.asarray(a, dtype=np.float32).astype(mybir.dt.np(BF16))

    # xET: x[src_e]^T, chunked along input dim -> [P, KIN*E1]
    xE = x[L1_src]  # [E1, IN_DIM]
    xET = np.ascontiguousarray(xE.T).reshape(KIN, P, E1)
    pk_x = bf(np.concatenate([xET[k] for k in range(KIN)], axis=1))

    # attA [F1, 2H] block-diagonal attention vectors, chunked -> [P, FH*2H]
    attA = np.zeros((F1, 2 * H), np.float32)
    for h in range(H):
        attA[h * Dh:(h + 1) * Dh, h] = a_src1[h]
        attA[h * Dh:(h + 1) * Dh, H + h] = a_dst1[h]
    attA = attA.reshape(FH, P, 2 * H)
    atta_pack = np.concatenate([attA[f] for f in range(FH)], axis=1)

    # dsel [E1, Sp] per edge-chunk (pad col gets a 1 in row 0 to keep den>0)
    dsel = np.zeros((E1, Sp), np.float32)
    e_ok = d1 >= 0
    dsel[np.arange(E1)[e_ok], d1[e_ok]] = 1.0
    if Sp > S:
        dsel[0, S:] = 1.0
    ech = [(0, E1)] if E1 == P else [(0, E1 - P), (E1 - P, E1)]
    NE = len(ech)
    dsel_pack = np.zeros((P, NE * H * Sp), np.float32)
    for i, (a, b) in enumerate(ech):
        for h in range(H):
            o = (i * H + h) * Sp
            dsel_pack[: b - a, o:o + Sp] = dsel[a:b]

    ident = np.eye(P, dtype=np.float32)
    a2both = np.zeros((P, 2), np.float32)  # [OUT, 2] = [a2s | a2d]
    a2both[:OUT, 0] = np.asarray(a_src2, np.float32).reshape(OUT)
    a2both[:OUT, 1] = np.asarray(a_dst2, np.float32).reshape(OUT)
    # misc [P, *] bf16 pack: attA | dsel | ident | a2both
    pk_m = bf(np.concatenate([atta_pack, dsel_pack, ident, a2both], axis=1))

    # w1 f-chunks (k-minor): one pack per f; last one also carries w2 chunks
    w1c = np.asarray(W1, np.float32).reshape(KIN, P, FH, P)
    wpk = []
    for f in range(FH):
        cols = [w1c[k, :, f, :] for k in range(KIN)]
        wpk.append(np.concatenate(cols, axis=1))
    w2c = np.asarray(W2, np.float32).reshape(FH, P, OUT)
    w2T = np.ascontiguousarray(np.asarray(W2, np.float32).T)  # [OUT, F1]
    pk_w2o = bf(np.concatenate([w2c[f] for f in range(FH)] + [w2T], axis=1))
    wpk = [bf(w) for w in wpk]

    # [S, *] bf16 pack: dselT | a2sb | a2db | sel2Tb
    dselT = np.ascontiguousarray(dsel[:, :S].T)  # [S, E1] true (no pad rows)
    a2sb = np.repeat(np.asarray(a_src2, np.float32).reshape(1, OUT), S, axis=0)
    a2db = np.repeat(np.asarray(a_dst2, np.float32).reshape(1, OUT), S, axis=0)
    sel2T = np.zeros((S, E2), np.float32)
    sel2T[s2, np.arange(E2)] = 1.0
    pk_s = bf(np.concatenate([dselT, a2sb, a2db, sel2T], axis=1))

    # [S, *] f32 pack: sel2Tf | d2Tf | b2 (row 0)
    d2T = np.zeros((S, E2), np.float32)
    d2T[s1pos[0], :] = 1.0
    b2pad = np.zeros((S, OUT), np.float32)
    b2pad[0] = np.asarray(b2, np.float32).reshape(OUT)
    pk_f = np.ascontiguousarray(np.concatenate([sel2T, d2T, b2pad], axis=1))

    pk_32 = np.ascontiguousarray(
        np.asarray(b1, np.float32).reshape(FH, P).T)  # [P, FH] f32

    dims = dict(E1=E1, S=S, Sp=Sp, E2=E2, KIN=KIN, FH=FH, H=H, Dh=Dh,
                IN_DIM=IN_DIM, OUT=OUT, NE=NE, ech=ech)
    arrs = dict(pk_x=np.ascontiguousarray(pk_x), pk_m=np.ascontiguousarray(pk_m),
                pk_s=np.ascontiguousarray(pk_s), pk_f=pk_f, pk_32=pk_32)
    for f in range(FH):
        arrs[f"pk_w{f}"] = np.ascontiguousarray(wpk[f])
    arrs["pk_w2o"] = np.ascontiguousarray(pk_w2o)
    return dims, arrs


def build_nc(d, shapes):
    E1, S, Sp, E2 = d["E1"], d["S"], d["Sp"], d["E2"]
    KIN, FH, H, OUT = d["KIN"], d["FH"], d["H"], d["OUT"]
    NE, ech = d["NE"], d["ech"]
    AF = mybir.ActivationFunctionType
    ALU = mybir.AluOpType

    nc = bacc.Bacc("TRN2", target_bir_lowering=False, debug=False,
                   num_devices=N_CORES)
    dram = {}
    for name in shapes:
        dt = F32 if name in ("pk_f", "pk_32") else BF16
        dram[name] = nc.dram_tensor(name, list(shapes[name]), dt,
                                    kind="ExternalInput").ap()
    out_d = nc.dram_tensor("out", [1, OUT], F32, kind="ExternalOutput").ap()

    with tile.TileContext(nc) as tc:
        with tc.tile_pool(name="sb", bufs=1) as sb, \
             tc.tile_pool(name="ps", bufs=1, space="PSUM") as ps:
            # ---- input DMAs, spread across queues ----
            def load(name, eng, dt=BF16):
                t = sb.tile(list(shapes[name]), dt, name=name + "_t")
                eng.dma_start(t[:, :], dram[name][:, :])
                return t

            pk_x = load("pk_x", nc.sync)      # SP (needed first)
            w0 = load("pk_w0", nc.gpsimd)     # Pool (SWDGE)
            w1_ = load("pk_w1", nc.sync)      # SP
            w2_ = load("pk_w2", nc.gpsimd)    # Pool
            w3 = load("pk_w3", nc.sync)       # SP (w1 f3)
            pk_m = load("pk_m", nc.scalar)    # Act (after table load)
            pks = load("pk_s", nc.scalar)
            pkf = load("pk_f", nc.scalar, F32)
            w2o = load("pk_w2o", nc.scalar)   # w2 chunks + w2T
            pk32 = load("pk_32", nc.scalar, F32)
            wtl = [w0, w1_, w2_, w3]

            # slices into the packs
            xet = [pk_x[:, k * E1:(k + 1) * E1] for k in range(KIN)]
            atta = [pk_m[:, f * 2 * H:(f + 1) * 2 * H] for f in range(FH)]
            o = FH * 2 * H
            dsel = [pk_m[: b - a, o + i * H * Sp: o + i * H * Sp + Sp]
                    for i, (a, b) in enumerate(ech)]
            dsel_cat = [pk_m[: b - a, o + i * H * Sp: o + (i + 1) * H * Sp]
                        for i, (a, b) in enumerate(ech)]
            o += NE * H * Sp
            ident = pk_m[:, o: o + P]
            a2both = pk_m[:, o + P: o + P + 2]
            w1sl = lambda f, k: wtl[f][:, k * P:(k + 1) * P]
            w2sl = [w2o[:, f * OUT:(f + 1) * OUT] for f in range(FH)]
            w2Tsl = [w2o[:, FH * OUT + f * P: FH * OUT + (f + 1) * P]
                     for f in range(FH)]
            dselT = pks[:, :E1]
            dselTc = [pks[:, a:b] for (a, b) in ech]
            a2sb = pks[:, E1: E1 + OUT]
            a2db = pks[:, E1 + OUT: E1 + 2 * OUT]
            sel2Tb = pks[:, E1 + 2 * OUT: E1 + 2 * OUT + E2]
            sel2Tf = pkf[:, :E2]
            d2Tf = pkf[:, E2: 2 * E2]
            b2row = pkf[0:1, 2 * E2: 2 * E2 + OUT]
            b1c = pk32

            # ---- phase 1: hET[f] = (x[src]@W1)^T chunks [P, E1], with the
            # per-edge src scores + alphaD matmuls interleaved (lag one f so
            # the PE never stalls on the PSUM->SBUF copies) ----
            sT_tiles = [ps.tile([b - a, H], F32, name=f"sT{i}", tag="attps",
                                bufs=2) for i, (a, b) in enumerate(ech)]
            sT_ps = [t[:, :] for t in sT_tiles]
            aDT_ps = ps.tile([S, H], F32, name="aDT_ps", tag="sm", bufs=2)
            hETs = [None] * FH
            hE = {}
            hE3_ps = {}

            def alpha_mms(f):
                for i, (a, b) in enumerate(ech):
                    nc.tensor.matmul(sT_ps[i], lhsT=hETs[f][:, a:b],
                                     rhs=atta[f][:, :H],
                                     start=(f == 0), stop=False,
                                     skip_group_check=True)
                nc.tensor.matmul(aDT_ps[:, :], lhsT=hETs[f][:, E1 - S:E1],
                                 rhs=atta[f][:, H:2 * H],
                                 start=(f == 0), stop=(f == FH - 1),
                                 skip_group_check=True)

            for f in range(FH):
                h_ps = ps.tile([P, E1], F32, name=f"hET{f}", tag="hps", bufs=2)
                for k in range(KIN):
                    nc.tensor.matmul(h_ps[:, :], lhsT=w1sl(f, k), rhs=xet[k],
                                     start=(k == 0), stop=(k == KIN - 1))
                if f > 0:
                    alpha_mms(f - 1)
                h_sb = sb.tile([P, E1], BF16, name=f"hETs{f}")
                nc.vector.tensor_copy(h_sb[:, :], h_ps[:, :])
                hETs[f] = h_sb
                for i, (a, b) in enumerate(ech):
                    # XBAR transpose needs a 128-wide source window; chunks
                    # are laid out so the needed rows start at partition 0.
                    # The last f-chunk lands too late for the 1.7us DMA
                    # latency -- use a PE transpose + DVE copy instead.
                    wb = max(b, a + P)
                    assert wb <= E1
                    if f == FH - 1:
                        n = b - a
                        t_ps = ps.tile([n, P], BF16, name=f"hEp{f}_{i}",
                                       tag="tp", bufs=2)
                        nc.tensor.transpose(t_ps[:, :], h_sb[:, a:b],
                                            ident[:, :])
                        hE3_ps[i] = t_ps
                    else:
                        t_sb = sb.tile([P, P], BF16, name=f"hE{f}_{i}")
                        eng = nc.sync if i == 0 else nc.scalar
                        eng.dma_start_transpose(t_sb[:, :], h_sb[:, wb - P:wb])
                        hE[(i, f)] = t_sb[: b - a, :]
            alpha_mms(FH - 1)
            aDT_sb = sb.tile([S, H], BF16, name="aDT_sb")
            nc.scalar.activation(aDT_sb[:, :], aDT_ps[:, :], AF.Identity)
            # scores += alpha_dst[dst_e]; then leaky+exp per chunk
            eeT = []
            for i, (a, b) in enumerate(ech):
                n = b - a
                nc.tensor.matmul(sT_ps[i], lhsT=dselTc[i],
                                 rhs=aDT_sb[:, :],
                                 start=False, stop=True, skip_group_check=True)
                sc_sb = sb.tile([n, H], F32, name=f"sSc{i}")
                if i == 0:
                    nc.vector.tensor_scalar_mul(sc_sb[:, :], sT_ps[i],
                                                NEG_SLOPE)
                else:
                    nc.scalar.activation(sc_sb[:, :], sT_ps[i], AF.Identity,
                                         scale=NEG_SLOPE)
                sl_sb = sb.tile([n, H], F32, name=f"sLc{i}")
                nc.vector.tensor_tensor(out=sl_sb[:, :], in0=sT_ps[i],
                                        in1=sc_sb[:, :], op=ALU.max)
                t_sb = sb.tile([n, H], BF16, name=f"eeTs{i}")
                nc.scalar.activation(t_sb[:, :], sl_sb[:, :], AF.Exp)
                eeT.append(t_sb)
            for i, (a, b) in enumerate(ech):
                t_sb = sb.tile([b - a, P], BF16, name=f"hE{FH - 1}_{i}")
                nc.vector.tensor_copy(t_sb[:, :], hE3_ps[i][:, :])
                hE[(i, FH - 1)] = t_sb

            pass
            den = ps.tile([Sp, H], F32, name="den", tag="sm", bufs=2)
            for i in range(NE):
                nc.tensor.matmul(den[:, :], lhsT=dsel[i], rhs=eeT[i][:, :],
                                 start=(i == 0), stop=(i == NE - 1))
            rden = sb.tile([Sp, H], F32, name="rden")
            nc.vector.reciprocal(rden[:, :], den[:, :])
            rden16 = sb.tile([Sp, H], BF16, name="rden16")
            nc.gpsimd.tensor_copy(rden16[:, :], rden[:, :])
            # wET = eeT * (1/den)[dst]; dselW[h] = dsel * wET[:,h]
            wET, dselW = [], {}
            for i, (a, b) in enumerate(ech):
                n = b - a
                r_ps = ps.tile([n, H], F32, name=f"dnE{i}", tag="sm", bufs=2)
                nc.tensor.matmul(r_ps[:, :], lhsT=dselTc[i],
                                 rhs=rden16[:S, :], start=True, stop=True)
                w_sb = sb.tile([n, H], F32, name=f"wET{i}")
                nc.vector.tensor_tensor(out=w_sb[:, :], in0=eeT[i][:, :],
                                        in1=r_ps[:, :], op=ALU.mult)
                wET.append(w_sb)
            for i, (a, b) in enumerate(ech):
                n = b - a
                for h in range(H):
                    w_sb = sb.tile([n, Sp], BF16, name=f"dWs{i}_{h}")
                    eng = nc.vector if h % 2 == 0 else nc.gpsimd
                    eng.tensor_scalar_mul(w_sb[:, :], dsel[i],
                                          wET[i][:, h:h + 1])
                    dselW[(i, h)] = w_sb
            c2 = []
            for f in range(FH):
                c_ps = ps.tile([P, 2], F32, name=f"c2_{f}", tag="tp", bufs=2)
                nc.tensor.matmul(c_ps[:, :], lhsT=w2Tsl[f], rhs=a2both,
                                 start=True, stop=True)
                c_sb = sb.tile([P, 2], BF16, name=f"c2s_{f}")
                nc.scalar.activation(c_sb[:, :], c_ps[:, :], AF.Identity)
                c2.append(c_sb)
            out1rT = []
            for f in range(FH):
                o_ps = ps.tile([P, Sp], F32, name=f"o1T{f}", tag="sm", bufs=2)
                for i in range(NE):
                    nc.tensor.matmul(o_ps[:, :], lhsT=hE[(i, f)],
                                     rhs=dselW[(i, f)],
                                     start=(i == 0), stop=(i == NE - 1))
                o_sb = sb.tile([P, Sp], BF16, name=f"o1rT{f}")
                if f % 2 == 0:
                    nc.vector.tensor_scalar(out=o_sb[:, :], in0=o_ps[:, :],
                                            scalar1=b1c[:, f:f + 1],
                                            scalar2=0.0, op0=ALU.add,
                                            op1=ALU.max)
                else:
                    nc.scalar.activation(o_sb[:, :], o_ps[:, :], AF.Relu,
                                         bias=b1c[:, f:f + 1])
                out1rT.append(o_sb)

            # ---- layer 2 ----
            g_ps = ps.tile([S, OUT], F32, name="g_ps", tag="hps", bufs=2)
            bT_ps = ps.tile([S, 2], F32, name="bT_ps", tag="hps", bufs=2)
            forder = list(range(FH))
            if FH == 4:
                forder = [0, 1, 3, 2]
            for j, f in enumerate(forder):
                nc.tensor.matmul(g_ps[:, :], lhsT=out1rT[f][:, :S], rhs=w2sl[f],
                                 start=(j == 0), stop=(j == FH - 1))
                nc.tensor.matmul(bT_ps[:, :], lhsT=out1rT[f][:, :S], rhs=c2[f],
                                 start=(j == 0), stop=(j == FH - 1))
            g_sb = sb.tile([S, OUT], BF16, name="g_sb")
            nc.vector.tensor_copy(g_sb[:, :], g_ps[:, :])
            bT_sb = sb.tile([S, 2], F32, name="bT_sb")
            nc.scalar.activation(bT_sb[:, :], bT_ps[:, :], AF.Identity)
            # gE = g[src2_e] rows (off critical path)
            gE_ps = ps.tile([E2, OUT], F32, name="gE_ps", tag="sm", bufs=2)
            nc.tensor.matmul(gE_ps[:, :], lhsT=sel2Tb[:, :], rhs=g_sb[:, :],
                             start=True, stop=True)
            gE_sb = sb.tile([E2, OUT], BF16, name="gE_sb")
            nc.vector.tensor_copy(gE_sb[:, :], gE_ps[:, :])
            # layer-2 scores as a column [E2,1]: exp output feeds fin directly
            s2_ps = ps.tile([E2, 1], F32, name="s2_ps", tag="sm", bufs=2)
            nc.tensor.matmul(s2_ps[:, :], lhsT=sel2Tf, rhs=bT_sb[:, 0:1],
                             start=True, stop=False)
            nc.tensor.matmul(s2_ps[:, :], lhsT=d2Tf, rhs=bT_sb[:, 1:2],
                             start=False, stop=True)
            s2c = sb.tile([E2, 1], F32, name="s2c")
            nc.scalar.activation(s2c[:, :], s2_ps[:, :], AF.Identity,
                                 scale=NEG_SLOPE)
            sL2 = sb.tile([E2, 1], F32, name="sL2")
            nc.vector.tensor_tensor(out=sL2[:, :], in0=s2_ps[:, :],
                                    in1=s2c[:, :], op=ALU.max)
            ee2c = sb.tile([E2, 1], BF16, name="ee2c")
            nc.scalar.activation(ee2c[:, :], sL2[:, :], AF.Exp)
            from concourse import bass_isa
            den2 = sb.tile([E2, 1], F32, name="den2")
            nc.gpsimd.partition_all_reduce(den2[:, :], ee2c[:, :], channels=E2,
                                           reduce_op=bass_isa.ReduceOp.add)
            r2 = sb.tile([1, 1], F32, name="r2")
            nc.vector.reciprocal(r2[:, :], den2[0:1, :])
            fin_ps = ps.tile([1, OUT], F32, name="fin_ps", tag="sm", bufs=2)
            nc.tensor.matmul(fin_ps[:, :], lhsT=ee2c[:, :], rhs=gE_sb[:, :],
                             start=True, stop=True)
            out_f = sb.tile([1, OUT], F32, name="out_f")
            nc.vector.scalar_tensor_tensor(
                out=out_f[:, :], in0=fin_ps[:, :], scalar=r2[:, :],
                in1=b2row, op0=ALU.mult, op1=ALU.add)
            nc.sync.dma_start(out_d[:, :], out_f[:, :])
    nc.compile()
    return nc


_RUN_KWARGS = {}


def kernel(x, edge_index, W1, a_src1, a_dst1, b1, W2, a_src2, a_dst2, b2):
    x = np.ascontiguousarray(np.asarray(x, dtype=np.float32))
    edge_index = np.asarray(edge_index, dtype=np.int32)
    d, arrs = build_data(x, edge_index, np.asarray(W1), np.asarray(a_src1),
                         np.asarray(a_dst1), np.asarray(b1), np.asarray(W2),
                         np.asarray(a_src2), np.asarray(a_dst2), np.asarray(b2))
    shapes = {k: v.shape for k, v in arrs.items()}
    nc = build_nc(d, shapes)
    in_maps = [dict(arrs) for _ in range(N_CORES)]
    res = run_bass_kernel_spmd(nc, in_maps, list(range(N_CORES)), **_RUN_KWARGS)
    out = res.results[0]["out"].reshape(d["OUT"]).astype(np.float32)
    kernel.last_results = res
    kernel.last_nc = nc
    kernel.last_in_maps = in_maps
    return out


# revision 51
# speedup vs baseline: 1.0079x; 1.0079x over previous
"""Trainium2 Bass kernel for nn_GATQueryProjector (2-layer GAT, output = node 0's row).

The reference returns only h[0] -- node 0's layer-2 GAT output. The exact
computation reduces to node 0's 2-hop neighborhood: E2 in-edges at layer 2
(dsts = node 0), whose sources S1 need layer-1 outputs, which need the E1
in-edges of S1. Host code does index work only (subgraph discovery, gathers,
packing); every NeuronCore runs the full floating-point computation
redundantly (no collectives -- the node feature table is "replicated" per the
sharding hint, and the pruned problem is tiny).

Device program (per core):
  hET[f,e]   = W1^T x[src_e]     24 bf16 matmuls, edge dim free (=E1), the
                                 per-edge score matmuls interleaved lag-one
  sT[e,h]    = attA gather       src scores direct; dst scores from the
                                 self-loop columns of hET + a dselT matmul
  softmax    = leaky/exp + 0/1-selection matmuls (den, gather) on the PE
  hE[e,f]    via XBAR transpose-DMAs (f0-f2, hidden under the GEMM) and a
                                 PE transpose for the late f3 chunk
  out1T[f,s] = sum_e w_e hE[e,f]; relu(+b1); g = out1 @ W2; layer-2
               attention over E2 edges; final weighted row + b2.
HW notes: gpsimd must stay SBUF-only; max one PSUM operand per DVE op;
no stride-0 broadcast APs; no divide ALU (reciprocal+mult instead).
"""

import numpy as np

import concourse.bacc as bacc
import concourse.mybir as mybir
import concourse.tile as tile
from concourse import bass
from concourse.bass_utils import run_bass_kernel_spmd

N_CORES = 8
NEG_SLOPE = 0.2
P = 128
BF16 = mybir.dt.bfloat16
F32 = mybir.dt.float32


def build_data(x, edge_index, W1, a_src1, a_dst1, b1, W2, a_src2, a_dst2, b2):
    """Host-side index work: node 0's 2-hop subgraph + packed device inputs."""
    x = np.asarray(x, dtype=np.float32)
    src0, dst0 = edge_index[0], edge_index[1]
    # layer-2 in-edges of node 0 (+ self-loop, as reference appends)
    e2_src = src0[dst0 == 0]
    L2_src = np.concatenate([e2_src, np.array([0], dtype=src0.dtype)])
    S1 = np.unique(L2_src)  # sorted 1-hop in-neighbors of 0 (incl 0)
    S = len(S1)
    # layer-1 in-edges of every v in S1 (+ self-loops, appended LAST in S1 order)
    m1 = np.isin(dst0, S1)
    u1, v1 = src0[m1], dst0[m1]
    L1_src = np.concatenate([u1, S1])
    L1_dst = np.concatenate([v1, S1])
    E1 = len(L1_src)
    E2 = len(L2_src)
    assert S <= 128 and E2 <= 128 and E1 <= 512, (S, E2, E1)
    padn = (P - E1 if E1 < P else E1 % 2)  # >=128 edges, even count
    if padn:
        L1_src = np.concatenate([L1_src, np.repeat(L1_src[-1:], padn)])
        L1_dst = np.concatenate(
            [L1_dst, np.full(padn, -1, dtype=L1_dst.dtype)])
        E1 += padn

    s1pos = {int(v): i for i, v in enumerate(S1)}
    d1 = np.array([s1pos.get(int(v), -1) for v in L1_dst])  # dst slot per edge
    s2 = np.array([s1pos[int(u)] for u in L2_src])  # src slot per layer-2 edge

    H, Dh = a_src1.shape
    F1 = H * Dh
    IN_DIM = x.shape[1]
    OUT = W2.shape[1]
    KIN = IN_DIM // P
    FH = F1 // P
    Sp = S + (S % 2)  # dsel free width (even)

    bf = lambda a: np.asarray(a, dtype=np.float32).astype(mybir.dt.np(BF16))

    # xET: x[src_e]^T, chunked along input dim -> [P, KIN*E1]
    xE = x[L1_src]  # [E1, IN_DIM]
    xET = np.ascontiguousarray(xE.T).reshape(KIN, P, E1)
    pk_x = bf(np.concatenate([xET[k] for k in range(KIN)], axis=1))

    # attA [F1, 2H] block-diagonal attention vectors, chunked -> [P, FH*2H]
    attA = np.zeros((F1, 2 * H), np.float32)
    for h in range(H):
        attA[h * Dh:(h + 1) * Dh, h] = a_src1[h]
        attA[h * Dh:(h + 1) * Dh, H + h] = a_dst1[h]
    attA = attA.reshape(FH, P, 2 * H)
    atta_pack = np.concatenate([attA[f] for f in range(FH)], axis=1)

    # dsel [E1, Sp] per edge-chunk (pad col gets a 1 in row 0 to keep den>0)
    dsel = np.zeros((E1, Sp), np.float32)
    e_ok = d1 >= 0
    dsel[np.arange(E1)[e_ok], d1[e_ok]] = 1.0
    if Sp > S:
        dsel[0, S:] = 1.0
    ech = [(0, E1)] if E1 == P else [(0, E1 - P), (E1 - P, E1)]
    NE = len(ech)
    dsel_pack = np.zeros((P, NE * H * Sp), np.float32)
    for i, (a, b) in enumerate(ech):
        for h in range(H):
            o = (i * H + h) * Sp
            dsel_pack[: b - a, o:o + Sp] = dsel[a:b]

    ident = np.eye(P, dtype=np.float32)
    a2both = np.zeros((P, 2), np.float32)  # [OUT, 2] = [a2s | a2d]
    a2both[:OUT, 0] = np.asarray(a_src2, np.float32).reshape(OUT)
    a2both[:OUT, 1] = np.asarray(a_dst2, np.float32).reshape(OUT)
    # misc [P, *] bf16 pack: attA | dsel | ident | a2both
    pk_m = bf(np.concatenate([atta_pack, dsel_pack, ident, a2both], axis=1))

    # w1 f-chunks (k-minor): one pack per f; last one also carries w2 chunks
    w1c = np.asarray(W1, np.float32).reshape(KIN, P, FH, P)
    wpk = []
    for f in range(FH):
        cols = [w1c[k, :, f, :] for k in range(KIN)]
        wpk.append(np.concatenate(cols, axis=1))
    w2c = np.asarray(W2, np.float32).reshape(FH, P, OUT)
    w2T = np.ascontiguousarray(np.asarray(W2, np.float32).T)  # [OUT, F1]
    pk_w2o = bf(np.concatenate([w2c[f] for f in range(FH)] + [w2T], axis=1))
    wpk = [bf(w) for w in wpk]

    # [S, *] bf16 pack: dselT | a2sb | a2db | sel2Tb
    dselT = np.ascontiguousarray(dsel[:, :S].T)  # [S, E1] true (no pad rows)
    a2sb = np.repeat(np.asarray(a_src2, np.float32).reshape(1, OUT), S, axis=0)
    a2db = np.repeat(np.asarray(a_dst2, np.float32).reshape(1, OUT), S, axis=0)
    sel2T = np.zeros((S, E2), np.float32)
    sel2T[s2, np.arange(E2)] = 1.0
    pk_s = bf(np.concatenate([dselT, a2sb, a2db, sel2T], axis=1))

    # [S, *] f32 pack: sel2Tf | d2Tf | b2 (row 0)
    d2T = np.zeros((S, E2), np.float32)
    d2T[s1pos[0], :] = 1.0
    b2pad = np.zeros((S, OUT), np.float32)
    b2pad[0] = np.asarray(b2, np.float32).reshape(OUT)
    pk_f = np.ascontiguousarray(
        np.concatenate([sel2T, d2T, b2pad, dselT], axis=1))

    pk_32 = np.ascontiguousarray(
        np.asarray(b1, np.float32).reshape(FH, P).T)  # [P, FH] f32

    dims = dict(E1=E1, S=S, Sp=Sp, E2=E2, KIN=KIN, FH=FH, H=H, Dh=Dh,
                IN_DIM=IN_DIM, OUT=OUT, NE=NE, ech=ech)
    arrs = dict(pk_x=np.ascontiguousarray(pk_x), pk_m=np.ascontiguousarray(pk_m),
                pk_s=np.ascontiguousarray(pk_s), pk_f=pk_f, pk_32=pk_32)
    for f in range(FH):
        arrs[f"pk_w{f}"] = np.ascontiguousarray(wpk[f])
    arrs["pk_w2o"] = np.ascontiguousarray(pk_w2o)
    return dims, arrs


def build_nc(d, shapes):
    E1, S, Sp, E2 = d["E1"], d["S"], d["Sp"], d["E2"]
    KIN, FH, H, OUT = d["KIN"], d["FH"], d["H"], d["OUT"]
    NE, ech = d["NE"], d["ech"]
    AF = mybir.ActivationFunctionType
    ALU = mybir.AluOpType

    nc = bacc.Bacc("TRN2", target_bir_lowering=False, debug=False,
                   num_devices=N_CORES)
    dram = {}
    for name in shapes:
        dt = F32 if name in ("pk_f", "pk_32") else BF16
        dram[name] = nc.dram_tensor(name, list(shapes[name]), dt,
                                    kind="ExternalInput").ap()
    out_d = nc.dram_tensor("out", [1, OUT], F32, kind="ExternalOutput").ap()

    with tile.TileContext(nc) as tc:
        with tc.tile_pool(name="sb", bufs=1) as sb, \
             tc.tile_pool(name="ps", bufs=1, space="PSUM") as ps:
            # ---- input DMAs, spread across queues ----
            def load(name, eng, dt=BF16):
                t = sb.tile(list(shapes[name]), dt, name=name + "_t")
                eng.dma_start(t[:, :], dram[name][:, :])
                return t

            pk_x = load("pk_x", nc.sync)      # SP (needed first)
            w0 = load("pk_w0", nc.gpsimd)     # Pool (SWDGE)
            w1_ = load("pk_w1", nc.sync)      # SP
            w2_ = load("pk_w2", nc.gpsimd)    # Pool
            w3 = load("pk_w3", nc.sync)       # SP (w1 f3)
            pk_m = load("pk_m", nc.scalar)    # Act (after table load)
            pks = load("pk_s", nc.scalar)
            pkf = load("pk_f", nc.scalar, F32)
            w2o = load("pk_w2o", nc.scalar)   # w2 chunks + w2T
            pk32 = load("pk_32", nc.scalar, F32)
            wtl = [w0, w1_, w2_, w3]

            # slices into the packs
            xet = [pk_x[:, k * E1:(k + 1) * E1] for k in range(KIN)]
            atta = [pk_m[:, f * 2 * H:(f + 1) * 2 * H] for f in range(FH)]
            o = FH * 2 * H
            dsel = [pk_m[: b - a, o + i * H * Sp: o + i * H * Sp + Sp]
                    for i, (a, b) in enumerate(ech)]
            dsel_cat = [pk_m[: b - a, o + i * H * Sp: o + (i + 1) * H * Sp]
                        for i, (a, b) in enumerate(ech)]
            o += NE * H * Sp
            ident = pk_m[:, o: o + P]
            a2both = pk_m[:, o + P: o + P + 2]
            w1sl = lambda f, k: wtl[f][:, k * P:(k + 1) * P]
            w2sl = [w2o[:, f * OUT:(f + 1) * OUT] for f in range(FH)]
            w2Tsl = [w2o[:, FH * OUT + f * P: FH * OUT + (f + 1) * P]
                     for f in range(FH)]
            dselT = pks[:, :E1]
            dselTc = [pks[:, a:b] for (a, b) in ech]
            a2sb = pks[:, E1: E1 + OUT]
            a2db = pks[:, E1 + OUT: E1 + 2 * OUT]
            sel2Tb = pks[:, E1 + 2 * OUT: E1 + 2 * OUT + E2]
            sel2Tf = pkf[:, :E2]
            d2Tf = pkf[:, E2: 2 * E2]
            b2row = pkf[0:1, 2 * E2: 2 * E2 + OUT]
            o = 2 * E2 + OUT
            dselTc32 = [pkf[:, o + a: o + b] for (a, b) in ech]
            b1c = pk32

            # ---- phase 1: hET[f] = (x[src]@W1)^T chunks [P, E1], with the
            # per-edge src scores + alphaD matmuls interleaved (lag one f so
            # the PE never stalls on the PSUM->SBUF copies) ----
            sT_tiles = [ps.tile([b - a, H], F32, name=f"sT{i}", tag="attps",
                                bufs=2) for i, (a, b) in enumerate(ech)]
            sT_ps = [t[:, :] for t in sT_tiles]
            aDT_ps = ps.tile([S, H], F32, name="aDT_ps", tag="sm", bufs=2)
            hETs = [None] * FH
            hE = {}
            hE3_ps = {}

            def alpha_mms(f):
                for i, (a, b) in enumerate(ech):
                    nc.tensor.matmul(sT_ps[i], lhsT=hETs[f][:, a:b],
                                     rhs=atta[f][:, :H],
                                     start=(f == 0), stop=False,
                                     skip_group_check=True)
                nc.tensor.matmul(aDT_ps[:, :], lhsT=hETs[f][:, E1 - S:E1],
                                 rhs=atta[f][:, H:2 * H],
                                 start=(f == 0), stop=(f == FH - 1),
                                 skip_group_check=True)

            for f in range(FH):
                h_ps = ps.tile([P, E1], F32, name=f"hET{f}", tag="hps", bufs=2)
                for k in range(KIN):
                    nc.tensor.matmul(h_ps[:, :], lhsT=w1sl(f, k), rhs=xet[k],
                                     start=(k == 0), stop=(k == KIN - 1))
                if f > 0:
                    alpha_mms(f - 1)
                h_sb = sb.tile([P, E1], BF16, name=f"hETs{f}")
                nc.vector.tensor_copy(h_sb[:, :], h_ps[:, :])
                hETs[f] = h_sb
                for i, (a, b) in enumerate(ech):
                    # XBAR transpose needs a 128-wide source window; chunks
                    # are laid out so the needed rows start at partition 0.
                    # The last f-chunk lands too late for the 1.7us DMA
                    # latency -- use a PE transpose + DVE copy instead.
                    wb = max(b, a + P)
                    assert wb <= E1
                    if f == FH - 1:
                        n = b - a
                        t_ps = ps.tile([n, P], BF16, name=f"hEp{f}_{i}",
                                       tag="tp", bufs=2)
                        nc.tensor.transpose(t_ps[:, :], h_sb[:, a:b],
                                            ident[:, :])
                        hE3_ps[i] = t_ps
                    else:
                        t_sb = sb.tile([P, P], BF16, name=f"hE{f}_{i}")
                        eng = nc.sync if i == 0 else nc.scalar
                        eng.dma_start_transpose(t_sb[:, :], h_sb[:, wb - P:wb])
                        hE[(i, f)] = t_sb[: b - a, :]
            alpha_mms(FH - 1)
            aDT_sb = sb.tile([S, H], BF16, name="aDT_sb")
            nc.scalar.activation(aDT_sb[:, :], aDT_ps[:, :], AF.Identity)
            # scores += alpha_dst[dst_e]; then leaky+exp per chunk
            eeT = []
            for i, (a, b) in enumerate(ech):
                n = b - a
                nc.tensor.matmul(sT_ps[i], lhsT=dselTc[i],
                                 rhs=aDT_sb[:, :],
                                 start=False, stop=True, skip_group_check=True)
                sc_sb = sb.tile([n, H], F32, name=f"sSc{i}")
                if i == 0:
                    nc.vector.tensor_scalar_mul(sc_sb[:, :], sT_ps[i],
                                                NEG_SLOPE)
                else:
                    nc.scalar.activation(sc_sb[:, :], sT_ps[i], AF.Identity,
                                         scale=NEG_SLOPE)
                sl_sb = sb.tile([n, H], F32, name=f"sLc{i}")
                nc.vector.tensor_tensor(out=sl_sb[:, :], in0=sT_ps[i],
                                        in1=sc_sb[:, :], op=ALU.max)
                t_sb = sb.tile([n, H], BF16, name=f"eeTs{i}")
                nc.scalar.activation(t_sb[:, :], sl_sb[:, :], AF.Exp)
                eeT.append(t_sb)
            for i, (a, b) in enumerate(ech):
                t_sb = sb.tile([b - a, P], BF16, name=f"hE{FH - 1}_{i}")
                nc.vector.tensor_copy(t_sb[:, :], hE3_ps[i][:, :])
                hE[(i, FH - 1)] = t_sb

            pass
            den = ps.tile([Sp, H], F32, name="den", tag="sm", bufs=2)
            for i in range(NE):
                nc.tensor.matmul(den[:, :], lhsT=dsel[i], rhs=eeT[i][:, :],
                                 start=(i == 0), stop=(i == NE - 1))
            rden = sb.tile([Sp, H], F32, name="rden")
            nc.vector.reciprocal(rden[:, :], den[:, :])
            # wET = eeT * (1/den)[dst]; dselW[h] = dsel * wET[:,h]
            wET, dselW = [], {}
            for i, (a, b) in enumerate(ech):
                n = b - a
                r_ps = ps.tile([n, H], F32, name=f"dnE{i}", tag="sm", bufs=2)
                nc.tensor.matmul(r_ps[:, :], lhsT=dselTc32[i],
                                 rhs=rden[:S, :], start=True, stop=True)
                w_sb = sb.tile([n, H], F32, name=f"wET{i}")
                nc.vector.tensor_tensor(out=w_sb[:, :], in0=eeT[i][:, :],
                                        in1=r_ps[:, :], op=ALU.mult)
                wET.append(w_sb)
            for i, (a, b) in enumerate(ech):
                n = b - a
                for h in range(H):
                    w_sb = sb.tile([n, Sp], BF16, name=f"dWs{i}_{h}")
                    eng = nc.vector if h % 2 == 0 else nc.gpsimd
                    eng.tensor_scalar_mul(w_sb[:, :], dsel[i],
                                          wET[i][:, h:h + 1])
                    dselW[(i, h)] = w_sb
            c2 = []
            for f in range(FH):
                c_ps = ps.tile([P, 2], F32, name=f"c2_{f}", tag="tp", bufs=2)
                nc.tensor.matmul(c_ps[:, :], lhsT=w2Tsl[f], rhs=a2both,
                                 start=True, stop=True)
                c_sb = sb.tile([P, 2], BF16, name=f"c2s_{f}")
                nc.scalar.activation(c_sb[:, :], c_ps[:, :], AF.Identity)
                c2.append(c_sb)
            out1rT = []
            for f in range(FH):
                o_ps = ps.tile([P, Sp], F32, name=f"o1T{f}", tag="sm", bufs=2)
                for i in range(NE):
                    nc.tensor.matmul(o_ps[:, :], lhsT=hE[(i, f)],
                                     rhs=dselW[(i, f)],
                                     start=(i == 0), stop=(i == NE - 1))
                o_sb = sb.tile([P, Sp], BF16, name=f"o1rT{f}")
                if f % 2 == 0:
                    nc.vector.tensor_scalar(out=o_sb[:, :], in0=o_ps[:, :],
                                            scalar1=b1c[:, f:f + 1],
                                            scalar2=0.0, op0=ALU.add,
                                            op1=ALU.max)
                else:
                    nc.scalar.activation(o_sb[:, :], o_ps[:, :], AF.Relu,
                                         bias=b1c[:, f:f + 1])
                out1rT.append(o_sb)

            # ---- layer 2 ----
            g_ps = ps.tile([S, OUT], F32, name="g_ps", tag="hps", bufs=2)
            bT_ps = ps.tile([S, 2], F32, name="bT_ps", tag="hps", bufs=2)
            forder = list(range(FH))
            if FH == 4:
                forder = [0, 1, 3, 2]
            for j, f in enumerate(forder):
                nc.tensor.matmul(g_ps[:, :], lhsT=out1rT[f][:, :S], rhs=w2sl[f],
                                 start=(j == 0), stop=(j == FH - 1))
                nc.tensor.matmul(bT_ps[:, :], lhsT=out1rT[f][:, :S], rhs=c2[f],
                                 start=(j == 0), stop=(j == FH - 1))
            g_sb = sb.tile([S, OUT], BF16, name="g_sb")
            nc.vector.tensor_copy(g_sb[:, :], g_ps[:, :])
            bT_sb = sb.tile([S, 2], F32, name="bT_sb")
            nc.scalar.activation(bT_sb[:, :], bT_ps[:, :], AF.Identity)
            # gE = g[src2_e] rows (off critical path)
            gE_ps = ps.tile([E2, OUT], F32, name="gE_ps", tag="sm", bufs=2)
            nc.tensor.matmul(gE_ps[:, :], lhsT=sel2Tb[:, :], rhs=g_sb[:, :],
                             start=True, stop=True)
            gE_sb = sb.tile([E2, OUT], BF16, name="gE_sb")
            nc.vector.tensor_copy(gE_sb[:, :], gE_ps[:, :])
            # layer-2 scores as a column [E2,1]: exp output feeds fin directly
            s2_ps = ps.tile([E2, 1], F32, name="s2_ps", tag="sm", bufs=2)
            nc.tensor.matmul(s2_ps[:, :], lhsT=sel2Tf, rhs=bT_sb[:, 0:1],
                             start=True, stop=False)
            nc.tensor.matmul(s2_ps[:, :], lhsT=d2Tf, rhs=bT_sb[:, 1:2],
                             start=False, stop=True)
            s2c = sb.tile([E2, 1], F32, name="s2c")
            nc.scalar.activation(s2c[:, :], s2_ps[:, :], AF.Identity,
                                 scale=NEG_SLOPE)
            sL2 = sb.tile([E2, 1], F32, name="sL2")
            nc.vector.tensor_tensor(out=sL2[:, :], in0=s2_ps[:, :],
                                    in1=s2c[:, :], op=ALU.max)
            ee2c = sb.tile([E2, 1], BF16, name="ee2c")
            nc.scalar.activation(ee2c[:, :], sL2[:, :], AF.Exp)
            from concourse import bass_isa
            den2 = sb.tile([E2, 1], F32, name="den2")
            nc.gpsimd.partition_all_reduce(den2[:, :], ee2c[:, :], channels=E2,
                                           reduce_op=bass_isa.ReduceOp.add)
            r2 = sb.tile([1, 1], F32, name="r2")
            nc.vector.reciprocal(r2[:, :], den2[0:1, :])
            fin_ps = ps.tile([1, OUT], F32, name="fin_ps", tag="sm", bufs=2)
            nc.tensor.matmul(fin_ps[:, :], lhsT=ee2c[:, :], rhs=gE_sb[:, :],
                             start=True, stop=True)
            out_f = sb.tile([1, OUT], F32, name="out_f")
            nc.vector.scalar_tensor_tensor(
                out=out_f[:, :], in0=fin_ps[:, :], scalar=r2[:, :],
                in1=b2row, op0=ALU.mult, op1=ALU.add)
            nc.sync.dma_start(out_d[:, :], out_f[:, :])
    nc.compile()
    return nc


_RUN_KWARGS = {}


def kernel(x, edge_index, W1, a_src1, a_dst1, b1, W2, a_src2, a_dst2, b2):
    x = np.ascontiguousarray(np.asarray(x, dtype=np.float32))
    edge_index = np.asarray(edge_index, dtype=np.int32)
    d, arrs = build_data(x, edge_index, np.asarray(W1), np.asarray(a_src1),
                         np.asarray(a_dst1), np.asarray(b1), np.asarray(W2),
                         np.asarray(a_src2), np.asarray(a_dst2), np.asarray(b2))
    shapes = {k: v.shape for k, v in arrs.items()}
    nc = build_nc(d, shapes)
    in_maps = [dict(arrs) for _ in range(N_CORES)]
    res = run_bass_kernel_spmd(nc, in_maps, list(range(N_CORES)), **_RUN_KWARGS)
    out = res.results[0]["out"].reshape(d["OUT"]).astype(np.float32)
    kernel.last_results = res
    kernel.last_nc = nc
    kernel.last_in_maps = in_maps
    return out


# revision 56
# speedup vs baseline: 1.0414x; 1.0333x over previous
"""Trainium2 Bass kernel for nn_GATQueryProjector (2-layer GAT, output = node 0's row).

The reference returns only h[0] -- node 0's layer-2 GAT output. The exact
computation reduces to node 0's 2-hop neighborhood: E2 in-edges at layer 2
(dsts = node 0), whose sources S1 need layer-1 outputs, which need the E1
in-edges of S1. Host code does index work only (subgraph discovery, gathers,
packing); every NeuronCore runs the full floating-point computation
redundantly (no collectives -- the node feature table is "replicated" per the
sharding hint, and the pruned problem is tiny).

Device program (per core):
  hET[f,e]   = W1^T x[src_e]     24 bf16 matmuls, edge dim free (=E1), the
                                 per-edge score matmuls interleaved lag-one
  sT[e,h]    = attA gather       src scores direct; dst scores from the
                                 self-loop columns of hET + a dselT matmul
  softmax    = leaky/exp + 0/1-selection matmuls (den, gather) on the PE
  hE[e,f]    via XBAR transpose-DMAs (f0-f2, hidden under the GEMM) and a
                                 PE transpose for the late f3 chunk
  out1T[f,s] = sum_e w_e hE[e,f]; relu(+b1); g = out1 @ W2; layer-2
               attention over E2 edges; final weighted row + b2.
HW notes: gpsimd must stay SBUF-only; max one PSUM operand per DVE op;
no stride-0 broadcast APs; no divide ALU (reciprocal+mult instead).
"""

import numpy as np

import concourse.bacc as bacc
import concourse.mybir as mybir
import concourse.tile as tile
from concourse import bass
from concourse.bass_utils import run_bass_kernel_spmd

N_CORES = 8
NEG_SLOPE = 0.2
P = 128
BF16 = mybir.dt.bfloat16
F32 = mybir.dt.float32


def build_data(x, edge_index, W1, a_src1, a_dst1, b1, W2, a_src2, a_dst2, b2):
    """Host-side index work: node 0's 2-hop subgraph + packed device inputs."""
    x = np.asarray(x, dtype=np.float32)
    src0, dst0 = edge_index[0], edge_index[1]
    # layer-2 in-edges of node 0 (+ self-loop, as reference appends)
    e2_src = src0[dst0 == 0]
    L2_src = np.concatenate([e2_src, np.array([0], dtype=src0.dtype)])
    S1 = np.unique(L2_src)  # sorted 1-hop in-neighbors of 0 (incl 0)
    S = len(S1)
    # layer-1 in-edges of every v in S1 (+ self-loops, appended LAST in S1 order)
    m1 = np.isin(dst0, S1)
    u1, v1 = src0[m1], dst0[m1]
    L1_src = np.concatenate([u1, S1])
    L1_dst = np.concatenate([v1, S1])
    E1 = len(L1_src)
    E2 = len(L2_src)
    assert S <= 128 and E2 <= 128 and E1 <= 512, (S, E2, E1)
    padn = (P - E1 if E1 < P else E1 % 2)  # >=128 edges, even count
    if padn:
        L1_src = np.concatenate([L1_src, np.repeat(L1_src[-1:], padn)])
        L1_dst = np.concatenate(
            [L1_dst, np.full(padn, -1, dtype=L1_dst.dtype)])
        E1 += padn

    s1pos = {int(v): i for i, v in enumerate(S1)}
    d1 = np.array([s1pos.get(int(v), -1) for v in L1_dst])  # dst slot per edge
    s2 = np.array([s1pos[int(u)] for u in L2_src])  # src slot per layer-2 edge

    H, Dh = a_src1.shape
    F1 = H * Dh
    IN_DIM = x.shape[1]
    OUT = W2.shape[1]
    KIN = IN_DIM // P
    FH = F1 // P
    Sp = S + (S % 2)  # dsel free width (even)

    bf = lambda a: np.asarray(a, dtype=np.float32).astype(mybir.dt.np(BF16))

    # xET: x[src_e]^T, chunked along input dim -> [P, KIN*E1]
    xE = x[L1_src]  # [E1, IN_DIM]
    xET = np.ascontiguousarray(xE.T).reshape(KIN, P, E1)
    pk_x = bf(np.concatenate([xET[k] for k in range(KIN)], axis=1))

    # attA [F1, 2H] block-diagonal attention vectors, chunked -> [P, FH*2H]
    attA = np.zeros((F1, 2 * H), np.float32)
    for h in range(H):
        attA[h * Dh:(h + 1) * Dh, h] = a_src1[h]
        attA[h * Dh:(h + 1) * Dh, H + h] = a_dst1[h]
    attA = attA.reshape(FH, P, 2 * H)
    atta_pack = np.concatenate([attA[f] for f in range(FH)], axis=1)

    # dsel [E1, Sp] per edge-chunk (pad col gets a 1 in row 0 to keep den>0)
    dsel = np.zeros((E1, Sp), np.float32)
    e_ok = d1 >= 0
    dsel[np.arange(E1)[e_ok], d1[e_ok]] = 1.0
    if Sp > S:
        dsel[0, S:] = 1.0
    ech = [(0, E1)] if E1 == P else [(0, E1 - P), (E1 - P, E1)]
    NE = len(ech)
    dsel_pack = np.zeros((P, NE * H * Sp), np.float32)
    for i, (a, b) in enumerate(ech):
        for h in range(H):
            o = (i * H + h) * Sp
            dsel_pack[: b - a, o:o + Sp] = dsel[a:b]

    ident = np.eye(P, dtype=np.float32)
    a2both = np.zeros((P, 2), np.float32)  # [OUT, 2] = [a2s | a2d]
    a2both[:OUT, 0] = np.asarray(a_src2, np.float32).reshape(OUT)
    a2both[:OUT, 1] = np.asarray(a_dst2, np.float32).reshape(OUT)
    # misc [P, *] bf16 pack: attA | dsel | ident | a2both
    pk_m = bf(np.concatenate([atta_pack, dsel_pack, ident, a2both], axis=1))

    # w1 f-chunks (k-minor): one pack per f; last one also carries w2 chunks
    w1c = np.asarray(W1, np.float32).reshape(KIN, P, FH, P)
    wpk = []
    for f in range(FH):
        cols = [w1c[k, :, f, :] for k in range(KIN)]
        wpk.append(np.concatenate(cols, axis=1))
    w2c = np.asarray(W2, np.float32).reshape(FH, P, OUT)
    w2T = np.ascontiguousarray(np.asarray(W2, np.float32).T)  # [OUT, F1]
    pk_w2o = bf(np.concatenate([w2c[f] for f in range(FH)] + [w2T], axis=1))
    wpk = [bf(w) for w in wpk]

    # [S, *] bf16 pack: dselT | a2sb | a2db | sel2Tb
    dselT = np.ascontiguousarray(dsel[:, :S].T)  # [S, E1] true (no pad rows)
    a2sb = np.repeat(np.asarray(a_src2, np.float32).reshape(1, OUT), S, axis=0)
    a2db = np.repeat(np.asarray(a_dst2, np.float32).reshape(1, OUT), S, axis=0)
    sel2T = np.zeros((S, E2), np.float32)
    sel2T[s2, np.arange(E2)] = 1.0
    pk_s = bf(np.concatenate([dselT, a2sb, a2db, sel2T], axis=1))

    # [S, *] f32 pack: sel2Tf | d2Tf | b2 (row 0)
    d2T = np.zeros((S, E2), np.float32)
    d2T[s1pos[0], :] = 1.0
    b2pad = np.zeros((S, OUT), np.float32)
    b2pad[0] = np.asarray(b2, np.float32).reshape(OUT)
    pk_f = np.ascontiguousarray(
        np.concatenate([sel2T, d2T, b2pad, dselT], axis=1))

    pk_32 = np.ascontiguousarray(
        np.asarray(b1, np.float32).reshape(FH, P).T)  # [P, FH] f32

    dims = dict(E1=E1, S=S, Sp=Sp, E2=E2, KIN=KIN, FH=FH, H=H, Dh=Dh,
                IN_DIM=IN_DIM, OUT=OUT, NE=NE, ech=ech)
    arrs = dict(pk_x=np.ascontiguousarray(pk_x), pk_m=np.ascontiguousarray(pk_m),
                pk_s=np.ascontiguousarray(pk_s), pk_f=pk_f, pk_32=pk_32)
    for f in range(FH):
        arrs[f"pk_w{f}"] = np.ascontiguousarray(wpk[f])
    arrs["pk_w2o"] = np.ascontiguousarray(pk_w2o)
    return dims, arrs


def build_nc(d, shapes):
    E1, S, Sp, E2 = d["E1"], d["S"], d["Sp"], d["E2"]
    KIN, FH, H, OUT = d["KIN"], d["FH"], d["H"], d["OUT"]
    NE, ech = d["NE"], d["ech"]
    AF = mybir.ActivationFunctionType
    ALU = mybir.AluOpType

    nc = bacc.Bacc("TRN2", target_bir_lowering=False, debug=False,
                   num_devices=N_CORES)
    dram = {}
    for name in shapes:
        dt = F32 if name in ("pk_f", "pk_32") else BF16
        dram[name] = nc.dram_tensor(name, list(shapes[name]), dt,
                                    kind="ExternalInput").ap()
    out_d = nc.dram_tensor("out", [1, OUT], F32, kind="ExternalOutput").ap()

    with tile.TileContext(nc) as tc:
        with tc.tile_pool(name="sb", bufs=1) as sb, \
             tc.tile_pool(name="ps", bufs=1, space="PSUM") as ps:
            # ---- input DMAs, spread across queues ----
            def load(name, eng, dt=BF16):
                t = sb.tile(list(shapes[name]), dt, name=name + "_t")
                eng.dma_start(t[:, :], dram[name][:, :])
                return t

            pk_x = load("pk_x", nc.sync)      # SP (needed first)
            w0 = load("pk_w0", nc.gpsimd)     # Pool (SWDGE)
            w1_ = load("pk_w1", nc.sync)      # SP
            w2_ = load("pk_w2", nc.gpsimd)    # Pool
            w3 = load("pk_w3", nc.sync)       # SP (w1 f3)
            pk_m = load("pk_m", nc.scalar)    # Act (after table load)
            pks = load("pk_s", nc.scalar)
            pkf = load("pk_f", nc.scalar, F32)
            w2o = load("pk_w2o", nc.scalar)   # w2 chunks + w2T
            pk32 = load("pk_32", nc.scalar, F32)
            wtl = [w0, w1_, w2_, w3]

            # slices into the packs
            xet = [pk_x[:, k * E1:(k + 1) * E1] for k in range(KIN)]
            atta = [pk_m[:, f * 2 * H:(f + 1) * 2 * H] for f in range(FH)]
            o = FH * 2 * H
            dsel = [pk_m[: b - a, o + i * H * Sp: o + i * H * Sp + Sp]
                    for i, (a, b) in enumerate(ech)]
            dsel_cat = [pk_m[: b - a, o + i * H * Sp: o + (i + 1) * H * Sp]
                        for i, (a, b) in enumerate(ech)]
            o += NE * H * Sp
            ident = pk_m[:, o: o + P]
            a2both = pk_m[:, o + P: o + P + 2]
            w1sl = lambda f, k: wtl[f][:, k * P:(k + 1) * P]
            w2sl = [w2o[:, f * OUT:(f + 1) * OUT] for f in range(FH)]
            w2Tsl = [w2o[:, FH * OUT + f * P: FH * OUT + (f + 1) * P]
                     for f in range(FH)]
            dselT = pks[:, :E1]
            dselTc = [pks[:, a:b] for (a, b) in ech]
            a2sb = pks[:, E1: E1 + OUT]
            a2db = pks[:, E1 + OUT: E1 + 2 * OUT]
            sel2Tb = pks[:, E1 + 2 * OUT: E1 + 2 * OUT + E2]
            sel2Tf = pkf[:, :E2]
            d2Tf = pkf[:, E2: 2 * E2]
            b2row = pkf[0:1, 2 * E2: 2 * E2 + OUT]
            o = 2 * E2 + OUT
            dselTc32 = [pkf[:, o + a: o + b] for (a, b) in ech]
            b1c = pk32

            # ---- phase 1: hET[f] = (x[src]@W1)^T chunks [P, E1], with the
            # per-edge src scores + alphaD matmuls interleaved (lag one f so
            # the PE never stalls on the PSUM->SBUF copies) ----
            sT_tiles = [ps.tile([b - a, H], F32, name=f"sT{i}", tag="attps",
                                bufs=2) for i, (a, b) in enumerate(ech)]
            sT_ps = [t[:, :] for t in sT_tiles]
            aDT_ps = ps.tile([S, H], F32, name="aDT_ps", tag="sm", bufs=2)
            hETs = [None] * FH
            hE = {}
            hE3_ps = {}

            def alpha_mms(f):
                for i, (a, b) in enumerate(ech):
                    nc.tensor.matmul(sT_ps[i], lhsT=hETs[f][:, a:b],
                                     rhs=atta[f][:, :H],
                                     start=(f == 0), stop=False,
                                     skip_group_check=True)
                nc.tensor.matmul(aDT_ps[:, :], lhsT=hETs[f][:, E1 - S:E1],
                                 rhs=atta[f][:, H:2 * H],
                                 start=(f == 0), stop=(f == FH - 1),
                                 skip_group_check=True)

            for f in range(FH):
                h_ps = ps.tile([P, E1], F32, name=f"hET{f}", tag="hps", bufs=2)
                for k in range(KIN):
                    nc.tensor.matmul(h_ps[:, :], lhsT=w1sl(f, k), rhs=xet[k],
                                     start=(k == 0), stop=(k == KIN - 1))
                if f > 0:
                    alpha_mms(f - 1)
                h_sb = sb.tile([P, E1], BF16, name=f"hETs{f}")
                nc.vector.tensor_copy(h_sb[:, :], h_ps[:, :])
                hETs[f] = h_sb
                for i, (a, b) in enumerate(ech):
                    # XBAR transpose needs a 128-wide source window; chunks
                    # are laid out so the needed rows start at partition 0.
                    # The last f-chunk lands too late for the 1.7us DMA
                    # latency -- use a PE transpose + DVE copy instead.
                    wb = max(b, a + P)
                    assert wb <= E1
                    if f == FH - 1:
                        n = b - a
                        t_ps = ps.tile([n, P], BF16, name=f"hEp{f}_{i}",
                                       tag="tp", bufs=2)
                        nc.tensor.transpose(t_ps[:, :], h_sb[:, a:b],
                                            ident[:, :])
                        hE3_ps[i] = t_ps
                    else:
                        t_sb = sb.tile([P, P], BF16, name=f"hE{f}_{i}")
                        eng = nc.sync if i == 0 else nc.scalar
                        eng.dma_start_transpose(t_sb[:, :], h_sb[:, wb - P:wb])
                        hE[(i, f)] = t_sb[: b - a, :]
            alpha_mms(FH - 1)
            aDT_sb = sb.tile([S, H], BF16, name="aDT_sb")
            nc.scalar.activation(aDT_sb[:, :], aDT_ps[:, :], AF.Identity)
            # scores += alpha_dst[dst_e]; then leaky+exp per chunk
            eeT = []
            for i, (a, b) in enumerate(ech):
                n = b - a
                nc.tensor.matmul(sT_ps[i], lhsT=dselTc[i],
                                 rhs=aDT_sb[:, :],
                                 start=False, stop=True, skip_group_check=True)
                sc_sb = sb.tile([n, H], F32, name=f"sSc{i}")
                if i == 0:
                    nc.vector.tensor_scalar_mul(sc_sb[:, :], sT_ps[i],
                                                NEG_SLOPE)
                else:
                    nc.scalar.activation(sc_sb[:, :], sT_ps[i], AF.Identity,
                                         scale=NEG_SLOPE)
                sl_sb = sb.tile([n, H], F32, name=f"sLc{i}")
                nc.vector.tensor_tensor(out=sl_sb[:, :], in0=sT_ps[i],
                                        in1=sc_sb[:, :], op=ALU.max)
                t_sb = sb.tile([n, H], BF16, name=f"eeTs{i}")
                nc.scalar.activation(t_sb[:, :], sl_sb[:, :], AF.Exp)
                eeT.append(t_sb)
            for i, (a, b) in enumerate(ech):
                t_sb = sb.tile([b - a, P], BF16, name=f"hE{FH - 1}_{i}")
                nc.vector.tensor_copy(t_sb[:, :], hE3_ps[i][:, :])
                hE[(i, FH - 1)] = t_sb

            pass
            den = ps.tile([Sp, H], F32, name="den", tag="sm", bufs=2)
            for i in range(NE):
                nc.tensor.matmul(den[:, :], lhsT=dsel[i], rhs=eeT[i][:, :],
                                 start=(i == 0), stop=(i == NE - 1))
            rden = sb.tile([Sp, H], F32, name="rden")
            nc.vector.reciprocal(rden[:, :], den[:, :])
            # wET = eeT * (1/den)[dst]; dselW[h] = dsel * wET[:,h]
            wET, dselW = [], {}
            for i, (a, b) in enumerate(ech):
                n = b - a
                r_ps = ps.tile([n, H], F32, name=f"dnE{i}", tag="sm", bufs=2)
                nc.tensor.matmul(r_ps[:, :], lhsT=dselTc32[i],
                                 rhs=rden[:S, :], start=True, stop=True)
                w_sb = sb.tile([n, H], F32, name=f"wET{i}")
                nc.vector.tensor_tensor(out=w_sb[:, :], in0=eeT[i][:, :],
                                        in1=r_ps[:, :], op=ALU.mult)
                wET.append(w_sb)
            for i, (a, b) in enumerate(ech):
                n = b - a
                for h in range(H):
                    w_sb = sb.tile([n, Sp], BF16, name=f"dWs{i}_{h}")
                    eng = nc.vector if h % 2 == 0 else nc.gpsimd
                    eng.tensor_scalar_mul(w_sb[:, :], dsel[i],
                                          wET[i][:, h:h + 1])
                    dselW[(i, h)] = w_sb
            c2 = []
            for f in range(FH):
                c_ps = ps.tile([P, 2], F32, name=f"c2_{f}", tag="attps", bufs=2)
                nc.tensor.matmul(c_ps[:, :], lhsT=w2Tsl[f], rhs=a2both,
                                 start=True, stop=True)
                c_sb = sb.tile([P, 2], BF16, name=f"c2s_{f}")
                nc.scalar.activation(c_sb[:, :], c_ps[:, :], AF.Identity)
                c2.append(c_sb)
            out1rT = []
            for f in range(FH):
                o_ps = ps.tile([P, Sp], F32, name=f"o1T{f}", tag="sm", bufs=2)
                for i in range(NE):
                    nc.tensor.matmul(o_ps[:, :], lhsT=hE[(i, f)],
                                     rhs=dselW[(i, f)],
                                     start=(i == 0), stop=(i == NE - 1))
                o_sb = sb.tile([P, Sp], BF16, name=f"o1rT{f}")
                if f % 2 == 0:
                    nc.vector.tensor_scalar(out=o_sb[:, :], in0=o_ps[:, :],
                                            scalar1=b1c[:, f:f + 1],
                                            scalar2=0.0, op0=ALU.add,
                                            op1=ALU.max)
                else:
                    nc.scalar.activation(o_sb[:, :], o_ps[:, :], AF.Relu,
                                         bias=b1c[:, f:f + 1])
                out1rT.append(o_sb)

            # ---- layer 2 ----
            g_ps = ps.tile([S, OUT], F32, name="g_ps", tag="hps", bufs=2)
            bT_ps = ps.tile([S, 2], F32, name="bT_ps", tag="hps", bufs=2)
            forder = list(range(FH))
            if FH == 4:
                forder = [0, 1, 3, 2]
            for j, f in enumerate(forder):
                nc.tensor.matmul(g_ps[:, :], lhsT=out1rT[f][:, :S], rhs=w2sl[f],
                                 start=(j == 0), stop=(j == FH - 1))
                nc.tensor.matmul(bT_ps[:, :], lhsT=out1rT[f][:, :S], rhs=c2[f],
                                 start=(j == 0), stop=(j == FH - 1))
            g_sb = sb.tile([S, OUT], BF16, name="g_sb")
            nc.vector.tensor_copy(g_sb[:, :], g_ps[:, :])
            bT_sb = sb.tile([S, 2], F32, name="bT_sb")
            nc.scalar.activation(bT_sb[:, :], bT_ps[:, :], AF.Identity)
            # gE = g[src2_e] rows (off critical path)
            gE_ps = ps.tile([E2, OUT], F32, name="gE_ps", tag="sm", bufs=2)
            nc.tensor.matmul(gE_ps[:, :], lhsT=sel2Tb[:, :], rhs=g_sb[:, :],
                             start=True, stop=True)
            gE_sb = sb.tile([E2, OUT], BF16, name="gE_sb")
            nc.vector.tensor_copy(gE_sb[:, :], gE_ps[:, :])
            # layer-2 scores as a column [E2,1]: exp output feeds fin directly
            s2_ps = ps.tile([E2, 1], F32, name="s2_ps", tag="sm", bufs=2)
            nc.tensor.matmul(s2_ps[:, :], lhsT=sel2Tf, rhs=bT_sb[:, 0:1],
                             start=True, stop=False)
            nc.tensor.matmul(s2_ps[:, :], lhsT=d2Tf, rhs=bT_sb[:, 1:2],
                             start=False, stop=True)
            s2c = sb.tile([E2, 1], F32, name="s2c")
            nc.scalar.activation(s2c[:, :], s2_ps[:, :], AF.Identity,
                                 scale=NEG_SLOPE)
            sL2 = sb.tile([E2, 1], F32, name="sL2")
            nc.vector.tensor_tensor(out=sL2[:, :], in0=s2_ps[:, :],
                                    in1=s2c[:, :], op=ALU.max)
            ee2c = sb.tile([E2, 1], BF16, name="ee2c")
            nc.scalar.activation(ee2c[:, :], sL2[:, :], AF.Exp)
            from concourse import bass_isa
            den2 = sb.tile([E2, 1], F32, name="den2")
            nc.gpsimd.partition_all_reduce(den2[:, :], ee2c[:, :], channels=E2,
                                           reduce_op=bass_isa.ReduceOp.add)
            r2 = sb.tile([1, 1], F32, name="r2")
            nc.vector.reciprocal(r2[:, :], den2[0:1, :])
            fin_ps = ps.tile([1, OUT], F32, name="fin_ps", tag="sm", bufs=2)
            nc.tensor.matmul(fin_ps[:, :], lhsT=ee2c[:, :], rhs=gE_sb[:, :],
                             start=True, stop=True)
            out_f = sb.tile([1, OUT], F32, name="out_f")
            nc.vector.scalar_tensor_tensor(
                out=out_f[:, :], in0=fin_ps[:, :], scalar=r2[:, :],
                in1=b2row, op0=ALU.mult, op1=ALU.add)
            nc.sync.dma_start(out_d[:, :], out_f[:, :])
    nc.compile()
    return nc


_RUN_KWARGS = {}


def kernel(x, edge_index, W1, a_src1, a_dst1, b1, W2, a_src2, a_dst2, b2):
    x = np.ascontiguousarray(np.asarray(x, dtype=np.float32))
    edge_index = np.asarray(edge_index, dtype=np.int32)
    d, arrs = build_data(x, edge_index, np.asarray(W1), np.asarray(a_src1),
                         np.asarray(a_dst1), np.asarray(b1), np.asarray(W2),
                         np.asarray(a_src2), np.asarray(a_dst2), np.asarray(b2))
    shapes = {k: v.shape for k, v in arrs.items()}
    nc = build_nc(d, shapes)
    in_maps = [dict(arrs) for _ in range(N_CORES)]
    res = run_bass_kernel_spmd(nc, in_maps, list(range(N_CORES)), **_RUN_KWARGS)
    out = res.results[0]["out"].reshape(d["OUT"]).astype(np.float32)
    kernel.last_results = res
    kernel.last_nc = nc
    kernel.last_in_maps = in_maps
    return out


# revision 60
# speedup vs baseline: 1.0429x; 1.0014x over previous
"""Trainium2 Bass kernel for nn_GATQueryProjector (2-layer GAT, output = node 0's row).

The reference returns only h[0] -- node 0's layer-2 GAT output. The exact
computation reduces to node 0's 2-hop neighborhood: E2 in-edges at layer 2
(dsts = node 0), whose sources S1 need layer-1 outputs, which need the E1
in-edges of S1. Host code does index work only (subgraph discovery, gathers,
packing); every NeuronCore runs the full floating-point computation
redundantly (no collectives -- the node feature table is "replicated" per the
sharding hint, and the pruned problem is tiny).

Device program (per core):
  hET[f,e]   = W1^T x[src_e]     24 bf16 matmuls, edge dim free (=E1), the
                                 per-edge score matmuls interleaved lag-one
  sT[e,h]    = attA gather       src scores direct; dst scores from the
                                 self-loop columns of hET + a dselT matmul
  softmax    = leaky/exp + 0/1-selection matmuls (den, gather) on the PE
  hE[e,f]    via XBAR transpose-DMAs (f0-f2, hidden under the GEMM) and a
                                 PE transpose for the late f3 chunk
  out1T[f,s] = sum_e w_e hE[e,f]; relu(+b1); g = out1 @ W2; layer-2
               attention over E2 edges; final weighted row + b2.
HW notes: gpsimd must stay SBUF-only; max one PSUM operand per DVE op;
no stride-0 broadcast APs; no divide ALU (reciprocal+mult instead).
"""

import numpy as np

import concourse.bacc as bacc
import concourse.mybir as mybir
import concourse.tile as tile
from concourse import bass
from concourse.bass_utils import run_bass_kernel_spmd

N_CORES = 8
NEG_SLOPE = 0.2
P = 128
BF16 = mybir.dt.bfloat16
F32 = mybir.dt.float32


def build_data(x, edge_index, W1, a_src1, a_dst1, b1, W2, a_src2, a_dst2, b2):
    """Host-side index work: node 0's 2-hop subgraph + packed device inputs."""
    x = np.asarray(x, dtype=np.float32)
    src0, dst0 = edge_index[0], edge_index[1]
    # layer-2 in-edges of node 0 (+ self-loop, as reference appends)
    e2_src = src0[dst0 == 0]
    L2_src = np.concatenate([e2_src, np.array([0], dtype=src0.dtype)])
    S1 = np.unique(L2_src)  # sorted 1-hop in-neighbors of 0 (incl 0)
    S = len(S1)
    # layer-1 in-edges of every v in S1 (+ self-loops, appended LAST in S1 order)
    m1 = np.isin(dst0, S1)
    u1, v1 = src0[m1], dst0[m1]
    L1_src = np.concatenate([u1, S1])
    L1_dst = np.concatenate([v1, S1])
    E1 = len(L1_src)
    E2 = len(L2_src)
    assert S <= 128 and E2 <= 128 and E1 <= 512, (S, E2, E1)
    padn = (P - E1 if E1 < P else E1 % 2)  # >=128 edges, even count
    if padn:
        L1_src = np.concatenate([L1_src, np.repeat(L1_src[-1:], padn)])
        L1_dst = np.concatenate(
            [L1_dst, np.full(padn, -1, dtype=L1_dst.dtype)])
        E1 += padn

    s1pos = {int(v): i for i, v in enumerate(S1)}
    d1 = np.array([s1pos.get(int(v), -1) for v in L1_dst])  # dst slot per edge
    s2 = np.array([s1pos[int(u)] for u in L2_src])  # src slot per layer-2 edge

    H, Dh = a_src1.shape
    F1 = H * Dh
    IN_DIM = x.shape[1]
    OUT = W2.shape[1]
    KIN = IN_DIM // P
    FH = F1 // P
    Sp = S + (S % 2)  # dsel free width (even)

    bf = lambda a: np.asarray(a, dtype=np.float32).astype(mybir.dt.np(BF16))

    # xET: x[src_e]^T, chunked along input dim -> [P, KIN*E1]
    xE = x[L1_src]  # [E1, IN_DIM]
    xET = np.ascontiguousarray(xE.T).reshape(KIN, P, E1)
    pk_x = bf(np.concatenate([xET[k] for k in range(KIN)], axis=1))

    # attA [F1, 2H] block-diagonal attention vectors, chunked -> [P, FH*2H]
    attA = np.zeros((F1, 2 * H), np.float32)
    for h in range(H):
        attA[h * Dh:(h + 1) * Dh, h] = a_src1[h]
        attA[h * Dh:(h + 1) * Dh, H + h] = a_dst1[h]
    attA = attA.reshape(FH, P, 2 * H)
    atta_pack = np.concatenate([attA[f] for f in range(FH)], axis=1)

    # dsel [E1, Sp] per edge-chunk (pad col gets a 1 in row 0 to keep den>0)
    dsel = np.zeros((E1, Sp), np.float32)
    e_ok = d1 >= 0
    dsel[np.arange(E1)[e_ok], d1[e_ok]] = 1.0
    if Sp > S:
        dsel[0, S:] = 1.0
    ech = [(0, E1)] if E1 == P else [(0, E1 - P), (E1 - P, E1)]
    NE = len(ech)
    dsel_pack = np.zeros((P, NE * H * Sp), np.float32)
    for i, (a, b) in enumerate(ech):
        for h in range(H):
            o = (i * H + h) * Sp
            dsel_pack[: b - a, o:o + Sp] = dsel[a:b]

    ident = np.eye(P, dtype=np.float32)
    a2both = np.zeros((P, 2), np.float32)  # [OUT, 2] = [a2s | a2d]
    a2both[:OUT, 0] = np.asarray(a_src2, np.float32).reshape(OUT)
    a2both[:OUT, 1] = np.asarray(a_dst2, np.float32).reshape(OUT)
    # misc [P, *] bf16 pack: attA | dsel | ident | a2both
    pk_m = bf(np.concatenate([atta_pack, dsel_pack, ident, a2both], axis=1))

    # w1 f-chunks (k-minor): one pack per f; last one also carries w2 chunks
    w1c = np.asarray(W1, np.float32).reshape(KIN, P, FH, P)
    wpk = []
    for f in range(FH):
        cols = [w1c[k, :, f, :] for k in range(KIN)]
        wpk.append(np.concatenate(cols, axis=1))
    w2c = np.asarray(W2, np.float32).reshape(FH, P, OUT)
    w2T = np.ascontiguousarray(np.asarray(W2, np.float32).T)  # [OUT, F1]
    pk_w2o = bf(np.concatenate([w2c[f] for f in range(FH)] + [w2T], axis=1))
    wpk = [bf(w) for w in wpk]

    # [S, *] bf16 pack: dselT | a2sb | a2db | sel2Tb
    dselT = np.ascontiguousarray(dsel[:, :S].T)  # [S, E1] true (no pad rows)
    a2sb = np.repeat(np.asarray(a_src2, np.float32).reshape(1, OUT), S, axis=0)
    a2db = np.repeat(np.asarray(a_dst2, np.float32).reshape(1, OUT), S, axis=0)
    sel2T = np.zeros((S, E2), np.float32)
    sel2T[s2, np.arange(E2)] = 1.0
    pk_s = bf(np.concatenate([dselT, a2sb, a2db, sel2T], axis=1))

    # [S, *] f32 pack: sel2Tf | d2Tf | b2 (row 0)
    d2T = np.zeros((S, E2), np.float32)
    d2T[s1pos[0], :] = 1.0
    b2pad = np.zeros((S, OUT), np.float32)
    b2pad[0] = np.asarray(b2, np.float32).reshape(OUT)
    pk_f = np.ascontiguousarray(
        np.concatenate([sel2T, d2T, b2pad, dselT], axis=1))

    pk_32 = np.ascontiguousarray(
        np.asarray(b1, np.float32).reshape(FH, P).T)  # [P, FH] f32

    dims = dict(E1=E1, S=S, Sp=Sp, E2=E2, KIN=KIN, FH=FH, H=H, Dh=Dh,
                IN_DIM=IN_DIM, OUT=OUT, NE=NE, ech=ech)
    arrs = dict(pk_x=np.ascontiguousarray(pk_x), pk_m=np.ascontiguousarray(pk_m),
                pk_s=np.ascontiguousarray(pk_s), pk_f=pk_f, pk_32=pk_32)
    for f in range(FH):
        arrs[f"pk_w{f}"] = np.ascontiguousarray(wpk[f])
    arrs["pk_w2o"] = np.ascontiguousarray(pk_w2o)
    return dims, arrs


def build_nc(d, shapes):
    E1, S, Sp, E2 = d["E1"], d["S"], d["Sp"], d["E2"]
    KIN, FH, H, OUT = d["KIN"], d["FH"], d["H"], d["OUT"]
    NE, ech = d["NE"], d["ech"]
    AF = mybir.ActivationFunctionType
    ALU = mybir.AluOpType

    nc = bacc.Bacc("TRN2", target_bir_lowering=False, debug=False,
                   num_devices=N_CORES)
    dram = {}
    for name in shapes:
        dt = F32 if name in ("pk_f", "pk_32") else BF16
        dram[name] = nc.dram_tensor(name, list(shapes[name]), dt,
                                    kind="ExternalInput").ap()
    out_d = nc.dram_tensor("out", [1, OUT], F32, kind="ExternalOutput").ap()

    with tile.TileContext(nc) as tc:
        with tc.tile_pool(name="sb", bufs=1) as sb, \
             tc.tile_pool(name="ps", bufs=1, space="PSUM") as ps:
            # ---- input DMAs, spread across queues ----
            def load(name, eng, dt=BF16):
                t = sb.tile(list(shapes[name]), dt, name=name + "_t")
                eng.dma_start(t[:, :], dram[name][:, :])
                return t

            pk_x = load("pk_x", nc.sync)      # SP (needed first)
            w0 = load("pk_w0", nc.gpsimd)     # Pool (SWDGE)
            w1_ = load("pk_w1", nc.sync)      # SP
            w2_ = load("pk_w2", nc.gpsimd)    # Pool
            w3 = load("pk_w3", nc.sync)       # SP (w1 f3)
            pk_m = load("pk_m", nc.scalar)    # Act (after table load)
            pks = load("pk_s", nc.scalar)
            pkf = load("pk_f", nc.scalar, F32)
            w2o = load("pk_w2o", nc.scalar)   # w2 chunks + w2T
            pk32 = load("pk_32", nc.scalar, F32)
            wtl = [w0, w1_, w2_, w3]

            # slices into the packs
            xet = [pk_x[:, k * E1:(k + 1) * E1] for k in range(KIN)]
            atta = [pk_m[:, f * 2 * H:(f + 1) * 2 * H] for f in range(FH)]
            o = FH * 2 * H
            dsel = [pk_m[: b - a, o + i * H * Sp: o + i * H * Sp + Sp]
                    for i, (a, b) in enumerate(ech)]
            dsel_cat = [pk_m[: b - a, o + i * H * Sp: o + (i + 1) * H * Sp]
                        for i, (a, b) in enumerate(ech)]
            o += NE * H * Sp
            ident = pk_m[:, o: o + P]
            a2both = pk_m[:, o + P: o + P + 2]
            w1sl = lambda f, k: wtl[f][:, k * P:(k + 1) * P]
            w2sl = [w2o[:, f * OUT:(f + 1) * OUT] for f in range(FH)]
            w2Tsl = [w2o[:, FH * OUT + f * P: FH * OUT + (f + 1) * P]
                     for f in range(FH)]
            dselT = pks[:, :E1]
            dselTc = [pks[:, a:b] for (a, b) in ech]
            a2sb = pks[:, E1: E1 + OUT]
            a2db = pks[:, E1 + OUT: E1 + 2 * OUT]
            sel2Tb = pks[:, E1 + 2 * OUT: E1 + 2 * OUT + E2]
            sel2Tf = pkf[:, :E2]
            d2Tf = pkf[:, E2: 2 * E2]
            b2row = pkf[0:1, 2 * E2: 2 * E2 + OUT]
            o = 2 * E2 + OUT
            dselTc32 = [pkf[:, o + a: o + b] for (a, b) in ech]
            b1c = pk32

            # ---- phase 1: hET[f] = (x[src]@W1)^T chunks [P, E1], with the
            # per-edge src scores + alphaD matmuls interleaved (lag one f so
            # the PE never stalls on the PSUM->SBUF copies) ----
            sT_tiles = [ps.tile([b - a, H], F32, name=f"sT{i}", tag="attps",
                                bufs=2) for i, (a, b) in enumerate(ech)]
            sT_ps = [t[:, :] for t in sT_tiles]
            aDT_ps = ps.tile([S, H], F32, name="aDT_ps", tag="sm", bufs=2)
            hETs = [None] * FH
            hE = {}
            hE3_ps = {}

            def alpha_mms(f):
                for i, (a, b) in enumerate(ech):
                    nc.tensor.matmul(sT_ps[i], lhsT=hETs[f][:, a:b],
                                     rhs=atta[f][:, :H],
                                     start=(f == 0), stop=False,
                                     skip_group_check=True)
                nc.tensor.matmul(aDT_ps[:, :], lhsT=hETs[f][:, E1 - S:E1],
                                 rhs=atta[f][:, H:2 * H],
                                 start=(f == 0), stop=(f == FH - 1),
                                 skip_group_check=True)

            for f in range(FH):
                h_ps = ps.tile([P, E1], F32, name=f"hET{f}", tag="hps", bufs=2)
                for k in range(KIN):
                    nc.tensor.matmul(h_ps[:, :], lhsT=w1sl(f, k), rhs=xet[k],
                                     start=(k == 0), stop=(k == KIN - 1))
                if f > 0:
                    alpha_mms(f - 1)
                h_sb = sb.tile([P, E1], BF16, name=f"hETs{f}")
                nc.vector.tensor_copy(h_sb[:, :], h_ps[:, :])
                hETs[f] = h_sb
                for i, (a, b) in enumerate(ech):
                    # XBAR transpose needs a 128-wide source window; chunks
                    # are laid out so the needed rows start at partition 0.
                    # The last f-chunk lands too late for the 1.7us DMA
                    # latency -- use a PE transpose + DVE copy instead.
                    wb = max(b, a + P)
                    assert wb <= E1
                    if f == FH - 1:
                        n = b - a
                        t_ps = ps.tile([n, P], BF16, name=f"hEp{f}_{i}",
                                       tag="tp", bufs=2)
                        nc.tensor.transpose(t_ps[:, :], h_sb[:, a:b],
                                            ident[:, :])
                        hE3_ps[i] = t_ps
                    else:
                        t_sb = sb.tile([P, P], BF16, name=f"hE{f}_{i}")
                        eng = nc.sync if i == 0 else nc.scalar
                        eng.dma_start_transpose(t_sb[:, :], h_sb[:, wb - P:wb])
                        hE[(i, f)] = t_sb[: b - a, :]
            alpha_mms(FH - 1)
            aDT_sb = sb.tile([S, H], BF16, name="aDT_sb")
            nc.scalar.activation(aDT_sb[:, :], aDT_ps[:, :], AF.Identity)
            # scores += alpha_dst[dst_e]; then leaky+exp per chunk
            eeT = []
            for i, (a, b) in enumerate(ech):
                n = b - a
                nc.tensor.matmul(sT_ps[i], lhsT=dselTc[i],
                                 rhs=aDT_sb[:, :],
                                 start=False, stop=True, skip_group_check=True)
                sc_sb = sb.tile([n, H], F32, name=f"sSc{i}")
                if i == 0:
                    nc.vector.tensor_scalar_mul(sc_sb[:, :], sT_ps[i],
                                                NEG_SLOPE)
                else:
                    nc.scalar.activation(sc_sb[:, :], sT_ps[i], AF.Identity,
                                         scale=NEG_SLOPE)
                sl_sb = sb.tile([n, H], F32, name=f"sLc{i}")
                nc.vector.tensor_tensor(out=sl_sb[:, :], in0=sT_ps[i],
                                        in1=sc_sb[:, :], op=ALU.max)
                t_sb = sb.tile([n, H], BF16, name=f"eeTs{i}")
                nc.scalar.activation(t_sb[:, :], sl_sb[:, :], AF.Exp)
                eeT.append(t_sb)
            for i, (a, b) in enumerate(ech):
                t_sb = sb.tile([b - a, P], BF16, name=f"hE{FH - 1}_{i}")
                nc.vector.tensor_copy(t_sb[:, :], hE3_ps[i][:, :])
                hE[(i, FH - 1)] = t_sb

            pass
            den = ps.tile([Sp, H], F32, name="den", tag="sm", bufs=2)
            for i in range(NE):
                nc.tensor.matmul(den[:, :], lhsT=dsel[i], rhs=eeT[i][:, :],
                                 start=(i == 0), stop=(i == NE - 1))
            rden = sb.tile([Sp, H], F32, name="rden")
            nc.vector.reciprocal(rden[:, :], den[:, :])
            # wET = eeT * (1/den)[dst]; dselW[h] = dsel * wET[:,h]
            wET, dselW = [], {}
            for i, (a, b) in enumerate(ech):
                n = b - a
                r_ps = ps.tile([n, H], F32, name=f"dnE{i}", tag="sm", bufs=2)
                nc.tensor.matmul(r_ps[:, :], lhsT=dselTc32[i],
                                 rhs=rden[:S, :], start=True, stop=True)
                w_sb = sb.tile([n, H], F32, name=f"wET{i}")
                nc.vector.tensor_tensor(out=w_sb[:, :], in0=eeT[i][:, :],
                                        in1=r_ps[:, :], op=ALU.mult)
                wET.append(w_sb)
            for i, (a, b) in enumerate(ech):
                n = b - a
                for h in range(H):
                    w_sb = sb.tile([n, Sp], BF16, name=f"dWs{i}_{h}")
                    eng = (nc.vector if (i == NE - 1 and h % 2 == 0)
                           else nc.gpsimd)
                    eng.tensor_scalar_mul(w_sb[:, :], dsel[i],
                                          wET[i][:, h:h + 1])
                    dselW[(i, h)] = w_sb
            c2 = []
            for f in range(FH):
                c_ps = ps.tile([P, 2], F32, name=f"c2_{f}", tag="attps", bufs=2)
                nc.tensor.matmul(c_ps[:, :], lhsT=w2Tsl[f], rhs=a2both,
                                 start=True, stop=True)
                c_sb = sb.tile([P, 2], BF16, name=f"c2s_{f}")
                nc.scalar.activation(c_sb[:, :], c_ps[:, :], AF.Identity)
                c2.append(c_sb)
            out1rT = []
            for f in range(FH):
                o_ps = ps.tile([P, Sp], F32, name=f"o1T{f}", tag="sm", bufs=2)
                for i in range(NE):
                    nc.tensor.matmul(o_ps[:, :], lhsT=hE[(i, f)],
                                     rhs=dselW[(i, f)],
                                     start=(i == 0), stop=(i == NE - 1))
                o_sb = sb.tile([P, Sp], BF16, name=f"o1rT{f}")
                if f % 2 == 0:
                    nc.vector.tensor_scalar(out=o_sb[:, :], in0=o_ps[:, :],
                                            scalar1=b1c[:, f:f + 1],
                                            scalar2=0.0, op0=ALU.add,
                                            op1=ALU.max)
                else:
                    nc.scalar.activation(o_sb[:, :], o_ps[:, :], AF.Relu,
                                         bias=b1c[:, f:f + 1])
                out1rT.append(o_sb)

            # ---- layer 2 ----
            g_ps = ps.tile([S, OUT], F32, name="g_ps", tag="hps", bufs=2)
            bT_ps = ps.tile([S, 2], F32, name="bT_ps", tag="hps", bufs=2)
            forder = list(range(FH))
            for j, f in enumerate(forder):
                nc.tensor.matmul(g_ps[:, :], lhsT=out1rT[f][:, :S], rhs=w2sl[f],
                                 start=(j == 0), stop=(j == FH - 1))
                nc.tensor.matmul(bT_ps[:, :], lhsT=out1rT[f][:, :S], rhs=c2[f],
                                 start=(j == 0), stop=(j == FH - 1))
            g_sb = sb.tile([S, OUT], BF16, name="g_sb")
            nc.vector.tensor_copy(g_sb[:, :], g_ps[:, :])
            bT_sb = sb.tile([S, 2], F32, name="bT_sb")
            nc.scalar.activation(bT_sb[:, :], bT_ps[:, :], AF.Identity)
            # gE = g[src2_e] rows (off critical path)
            gE_ps = ps.tile([E2, OUT], F32, name="gE_ps", tag="sm", bufs=2)
            nc.tensor.matmul(gE_ps[:, :], lhsT=sel2Tb[:, :], rhs=g_sb[:, :],
                             start=True, stop=True)
            gE_sb = sb.tile([E2, OUT], BF16, name="gE_sb")
            nc.vector.tensor_copy(gE_sb[:, :], gE_ps[:, :])
            # layer-2 scores as a column [E2,1]: exp output feeds fin directly
            s2_ps = ps.tile([E2, 1], F32, name="s2_ps", tag="sm", bufs=2)
            nc.tensor.matmul(s2_ps[:, :], lhsT=sel2Tf, rhs=bT_sb[:, 0:1],
                             start=True, stop=False)
            nc.tensor.matmul(s2_ps[:, :], lhsT=d2Tf, rhs=bT_sb[:, 1:2],
                             start=False, stop=True)
            s2c = sb.tile([E2, 1], F32, name="s2c")
            nc.vector.tensor_scalar_mul(s2c[:, :], s2_ps[:, :], NEG_SLOPE)
            sL2 = sb.tile([E2, 1], F32, name="sL2")
            nc.vector.tensor_tensor(out=sL2[:, :], in0=s2_ps[:, :],
                                    in1=s2c[:, :], op=ALU.max)
            ee2c = sb.tile([E2, 1], BF16, name="ee2c")
            nc.scalar.activation(ee2c[:, :], sL2[:, :], AF.Exp)
            from concourse import bass_isa
            den2 = sb.tile([E2, 1], F32, name="den2")
            nc.gpsimd.partition_all_reduce(den2[:, :], ee2c[:, :], channels=E2,
                                           reduce_op=bass_isa.ReduceOp.add)
            r2 = sb.tile([1, 1], F32, name="r2")
            nc.vector.reciprocal(r2[:, :], den2[0:1, :])
            fin_ps = ps.tile([1, OUT], F32, name="fin_ps", tag="sm", bufs=2)
            nc.tensor.matmul(fin_ps[:, :], lhsT=ee2c[:, :], rhs=gE_sb[:, :],
                             start=True, stop=True)
            out_f = sb.tile([1, OUT], F32, name="out_f")
            nc.vector.scalar_tensor_tensor(
                out=out_f[:, :], in0=fin_ps[:, :], scalar=r2[:, :],
                in1=b2row, op0=ALU.mult, op1=ALU.add)
            nc.sync.dma_start(out_d[:, :], out_f[:, :])
    nc.compile()
    return nc


_RUN_KWARGS = {}


def kernel(x, edge_index, W1, a_src1, a_dst1, b1, W2, a_src2, a_dst2, b2):
    x = np.ascontiguousarray(np.asarray(x, dtype=np.float32))
    edge_index = np.asarray(edge_index, dtype=np.int32)
    d, arrs = build_data(x, edge_index, np.asarray(W1), np.asarray(a_src1),
                         np.asarray(a_dst1), np.asarray(b1), np.asarray(W2),
                         np.asarray(a_src2), np.asarray(a_dst2), np.asarray(b2))
    shapes = {k: v.shape for k, v in arrs.items()}
    nc = build_nc(d, shapes)
    in_maps = [dict(arrs) for _ in range(N_CORES)]
    res = run_bass_kernel_spmd(nc, in_maps, list(range(N_CORES)), **_RUN_KWARGS)
    out = res.results[0]["out"].reshape(d["OUT"]).astype(np.float32)
    kernel.last_results = res
    kernel.last_nc = nc
    kernel.last_in_maps = in_maps
    return out
